# revision 21
# baseline (speedup 1.0000x reference)
"""Longformer-style windowed self-attention for TRN2, 8-core SPMD.

Sharding: 24 (batch, head) pairs -> 3 heads per core (core c gets batch c//4,
heads (c%4)*3 .. +3). Each core computes QKV projections for its head slice,
windowed attention (block 256, window +-256), and writes its [4096, 192]
output channel slice. Host gathers slices into the full [2, 4096, 768] output.

All matmul inputs are bf16 (inputs/weights converted on host). Scores are
computed transposed ([keys, queries]); probs (exp'd scores) become the
stationary operand of the PV matmul, which therefore produces output directly
in [queries, head_dim] layout with a ones-column carrying the softmax
denominator - no PE transposes needed. Band-mask multiplies run on GpSimd,
exp on the scalar engine, PSUM evacuation + normalize scaling on DVE.
"""

import sys

for _p in ("/opt/trn_rl_repo", "/opt/pypackages"):
    if _p not in sys.path:
        sys.path.append(_p)

import numpy as np
import ml_dtypes
from contextlib import ExitStack

import concourse.bass as bass
import concourse.bacc as bacc
import concourse.mybir as mybir
import concourse.tile as tile
from concourse.bass_utils import run_bass_kernel_spmd

F32 = mybir.dt.float32
BF16 = mybir.dt.bfloat16
EXP = mybir.ActivationFunctionType.Exp
MUL = mybir.AluOpType.mult

B, S, D = 2, 4096, 768
H, DH = 12, 64
W = 256                 # one-sided window / query block size
NB = S // W             # 16 query blocks
NKC = S // 128          # 32 key chunks of 128
HPC = 3                 # heads per core
N_CORES = 8


def block_layout(n):
    """Score-PSUM column layout for query block n.

    Returns (pieces, maskop, ncols). pieces = [(m, qlo, qhi, col)]: key chunk
    m's scores for local queries [qlo, qhi) live at psum cols [col, col+qhi-qlo).
    maskop = (dst_col, width, src_col) multiplies pt[:, dst:dst+width] by
    msk[:, src:src+width] (msk = [L|L|U|U]). 256-wide pieces sit at byte
    offsets that never straddle a 2KB PSUM bank.
    """
    if n == 0:
        pieces = [(0, 0, 256, 0), (1, 0, 256, 256),
                  (3, 128, 256, 512), (2, 0, 256, 640)]
        maskop = (512, 256, 256)  # [mR2 | mR1 tri] *= [U|U]
        ncols = 896
    elif n == NB - 1:
        m0 = 2 * n
        pieces = [(m0, 0, 256, 0), (m0 - 1, 0, 256, 256),
                  (m0 - 2, 0, 128, 512), (m0 + 1, 0, 256, 640)]
        maskop = (384, 256, 0)    # [mL1 tri | mL2] *= [L|L]
        ncols = 896
    else:
        pieces = [(2 * n - 1, 0, 256, 0), (2 * n - 2, 0, 128, 256),
                  (2 * n + 3, 128, 256, 384), (2 * n + 2, 0, 256, 512),
                  (2 * n, 0, 256, 768), (2 * n + 1, 0, 256, 1024)]
        maskop = (128, 512, 0)    # [mL1 tri | mL2 | mR2 | mR1 tri] *= [L|L|U|U]
        ncols = 1280
    return pieces, maskop, ncols


def pv_chunks(pieces, half):
    """(m, pt_col) for key chunks fully covering query half [128h, 128h+128)."""
    q0, q1 = 128 * half, 128 * half + 128
    return [(m, col + q0 - qlo) for (m, qlo, qhi, col) in pieces
            if qlo <= q0 and q1 <= qhi]


def build_program(has_bias, has_kmask):
    nc = bacc.Bacc("TRN2", target_bir_lowering=False, debug=False,
                   num_devices=N_CORES)
    hsT_d = nc.declare_dram_parameter("hsT", [D, S], BF16, isOutput=False)
    w_d = nc.declare_dram_parameter("wqkv", [D, 576], BF16, isOutput=False)
    msk_d = nc.declare_dram_parameter("masks", [128, 512], BF16, isOutput=False)
    if has_bias:
        bqkv_d = nc.declare_dram_parameter("bqkv", [1, 576], BF16, isOutput=False)
    if has_kmask:
        kpad_d = nc.declare_dram_parameter("kpad", [128, NKC], F32, isOutput=False)
        qpad_d = nc.declare_dram_parameter("qpad", [128, NKC], F32, isOutput=False)
    out_d = nc.declare_dram_parameter("out", [S, HPC * DH], F32, isOutput=True)

    with tile.TileContext(nc) as tc, ExitStack() as ctx:
        const_p = ctx.enter_context(tc.tile_pool(name="const", bufs=1))
        hst_p = ctx.enter_context(tc.tile_pool(name="hst", bufs=3))
        qkt_p = ctx.enter_context(tc.tile_pool(name="qkt", bufs=1))
        vall_p = ctx.enter_context(tc.tile_pool(name="vall", bufs=1))
        pt_p = ctx.enter_context(tc.tile_pool(name="pt", bufs=4))
        wk_p = ctx.enter_context(tc.tile_pool(name="wk", bufs=4))
        ps_p = ctx.enter_context(tc.tile_pool(name="ps", bufs=2, space="PSUM"))
        sm_p = ctx.enter_context(tc.tile_pool(name="sm", bufs=2, space="PSUM"))

        # ---- constants / weights ----
        wsb = const_p.tile([128, 6, 576], BF16)
        w_r = w_d[:].rearrange("(c p) n -> p c n", p=128)

        hst_tiles = {}

        def dma_hst(t):
            hst = hst_p.tile([128, 6, 512], BF16)
            hst_tiles[t] = hst
            src = hsT_d[:].rearrange("(c p) s -> p c s", p=128)[
                :, :, 512 * t : 512 * t + 512
            ]
            if t == 0:  # split so the first projection group starts sooner
                nc.sync.dma_start(hst[:, 0:2, :], src[:, 0:2, :])
                nc.sync.dma_start(hst[:, 2:4, :], src[:, 2:4, :])
                nc.sync.dma_start(hst[:, 4:6, :], src[:, 4:6, :])
            else:
                nc.sync.dma_start(hst[:], src)

        nc.sync.dma_start(wsb[:, :, 0:128], w_r[:, :, 0:128])
        dma_hst(0)
        nc.sync.dma_start(wsb[:, :, 128:576], w_r[:, :, 128:576])
        msk_sb = const_p.tile([128, 512], BF16)
        nc.sync.dma_start(msk_sb[:], msk_d[:, :])
        dma_hst(1)
        if has_bias:
            bqkv_sb = const_p.tile([1, 576], BF16)
            nc.sync.dma_start(bqkv_sb[:], bqkv_d[:, :])
            ones_sb = const_p.tile([1, 512], BF16)
            nc.vector.memset(ones_sb[:], 1.0)
        if has_kmask:
            kpad_sb = const_p.tile([128, NKC], F32)
            nc.sync.dma_start(kpad_sb[:], kpad_d[:, :])
            qpad_sb = const_p.tile([128, NKC], F32)
            nc.sync.dma_start(qpad_sb[:], qpad_d[:, :])

        # PE warmup: dummy matmuls keep the tensor engine "busy" while the
        # first DMAs land, so the p-state ramp hits full clock before real
        # matmuls start. Inputs are never-written scratch; output is the
        # first sm-pool psum tile, freed immediately (no readers).
        warm_sb = const_p.tile([1, 512], BF16)
        nc.vector.memset(warm_sb[:], 0.0)
        warm_ps = sm_p.tile([128, 512], F32, space="PSUM", tag="sm")
        for _ in range(10):
            nc.tensor.matmul(
                warm_ps[:], warm_sb[0:1, 0:128], warm_sb[0:1, :],
                start=True, stop=True,
            )

        # qT/kT for head pair (A,B): A on partitions 0:64, B on 64:128
        qt_ab = qkt_p.tile([128, S], BF16)
        kt_ab = qkt_p.tile([128, S], BF16)
        # solo head C: base-0 tiles
        qt_c = qkt_p.tile([64, S], BF16)
        kt_c = qkt_p.tile([64, S], BF16)
        # v in [key, dh] layout: [128, key-chunk, (vA|1|vB|1|vC|1)]
        vall = vall_p.tile([128, NKC, 195], BF16)
        ones_cols = vall[:].rearrange("p m (h x) -> p m h x", h=3)[:, :, :, 64:65]
        nc.vector.memset(ones_cols, 1.0)

        def emit_proj_qk(t):
            s0 = 512 * t
            if t + 1 < 8:
                dma_hst(t + 1)
            hst = hst_tiles[t]
            for j in range(3):
                pp = sm_p.tile([128, 512], F32, space="PSUM", tag="sm")
                for c in range(6):
                    nc.tensor.matmul(
                        pp[:],
                        (wsb[:, c, 128 * j : 128 * j + 128]),
                        (hst[:, c, :]),
                        start=(c == 0),
                        stop=(c == 5 and not has_bias),
                    )
                if has_bias:
                    nc.tensor.matmul(
                        pp[:],
                        (bqkv_sb[0:1, 128 * j : 128 * j + 128]),
                        (ones_sb[0:1, :]),
                        start=False,
                        stop=True,
                    )
                if j == 0:
                    nc.vector.tensor_copy(qt_ab[:, s0 : s0 + 512], pp[:])
                elif j == 1:
                    nc.vector.tensor_copy(kt_ab[:, s0 : s0 + 512], pp[:])
                else:
                    nc.vector.tensor_copy(qt_c[:, s0 : s0 + 512], pp[0:64, :])
                    kcs = wk_p.tile([128, 512], BF16, name="kcs")
                    nc.vector.tensor_copy(kcs[64:128, :], pp[64:128, :])
                    nc.sync.dma_start(kt_c[:, s0 : s0 + 512], kcs[64:128, :])

        def emit_proj_v(t, groups=(0, 2), done=True):
            hst = hst_tiles.pop(t) if done else hst_tiles[t]
            for mm0 in groups:
                m = 4 * t + mm0
                pv = sm_p.tile([128, 512], F32, space="PSUM", tag="sm")
                for half, mm in enumerate((mm0, mm0 + 1)):
                    for c in range(6):
                        nc.tensor.matmul(
                            pv[:, 256 * half : 256 * half + 192],
                            (hst[:, c, 128 * mm : 128 * mm + 128]),
                            (wsb[:, c, 384:576]),
                            start=(c == 0),
                            stop=(c == 5 and not has_bias),
                        )
                    if has_bias:
                        nc.tensor.matmul(
                            pv[:, 256 * half : 256 * half + 192],
                            (ones_sb[0:1, 0:128]),
                            (bqkv_sb[0:1, 384:576]),
                            start=False,
                            stop=True,
                        )
                dst = vall[:, m : m + 2, :].rearrange(
                    "p m (h x) -> p m h x", h=3
                )[:, :, :, 0:64]
                src = pv[:].rearrange("p (m x) -> p m x", m=2)[
                    :, :, 0:192
                ].rearrange("p m (h x) -> p m h x", h=3)
                nc.vector.tensor_copy(dst, src)

        HEADS = (
            (lambda: kt_ab[0:64, :], lambda: qt_ab[0:64, :]),
            (lambda: kt_ab[64:128, :], lambda: qt_ab[64:128, :]),
            (lambda: kt_c[:, :], lambda: qt_c[:, :]),
        )

        def emit_block(n):
            pieces, maskop, ncols = block_layout(n)
            q0 = 256 * n
            pts = []
            for h, (ktf, qtf) in enumerate(HEADS):
                kt, qt = ktf(), qtf()
                ps = ps_p.tile([128, 1280], F32, space="PSUM", tag="ps")
                for m, qlo, qhi, col in pieces:
                    nc.tensor.matmul(
                        ps[:, col : col + qhi - qlo],
                        (kt[:, 128 * m : 128 * m + 128]),
                        (qt[:, q0 + qlo : q0 + qhi]),
                        start=True,
                        stop=True,
                    )
                pt = pt_p.tile([128, 1280], BF16, tag="pt")
                pts.append(pt)
                nc.scalar.activation(pt[:, 0:ncols], ps[:, 0:ncols], EXP)
                dcol, width, scol = maskop
                nc.vector.scalar_tensor_tensor(
                    pt[:, dcol : dcol + width],
                    pt[:, dcol : dcol + width],
                    1.0,
                    msk_sb[:, scol : scol + width],
                    MUL,
                    MUL,
                )
                if has_kmask:
                    for m, qlo, qhi, col in pieces:
                        nc.vector.tensor_scalar_mul(
                            pt[:, col : col + qhi - qlo],
                            pt[:, col : col + qhi - qlo],
                            kpad_sb[:, m : m + 1],
                        )

            # PV: out[q, dh] = pt(chunk).T @ [v|1]; col 64 of each head's rhs
            # slice is the ones column carrying the softmax denominator.
            outp = ps_p.tile([128, 1280], F32, space="PSUM", tag="ps")
            dcol, width, _ = maskop
            for h, pt in enumerate(pts):
                for half in (0, 1):
                    chunks = pv_chunks(pieces, half)
                    # unmasked chunks first: their matmuls only depend on the
                    # exp, so PV starts while the mask op is still running
                    chunks.sort(
                        key=lambda mp: not (
                            mp[1] + 128 <= dcol or mp[1] >= dcol + width
                        )
                    )
                    for ci, (m, pcol) in enumerate(chunks):
                        nc.tensor.matmul(
                            outp[:, 256 * half + 65 * h : 256 * half + 65 * h + 65],
                            (pt[:, pcol : pcol + 128]),
                            (vall[:, m, 65 * h : 65 * h + 65]),
                            start=(ci == 0),
                            stop=(ci == len(chunks) - 1),
                        )

            rec = wk_p.tile([128, 8], F32, name="rec")
            osb = wk_p.tile([128, 2, 192], F32, name="osb")
            for half in (0, 1):
                dcols = outp[:, 256 * half : 256 * half + 195].rearrange(
                    "p (i x) -> p i x", x=65
                )[:, :, 64:65]
                nc.vector.reciprocal(
                    rec[:, 4 * half : 4 * half + 3].rearrange(
                        "p (i x) -> p i x", x=1
                    ),
                    dcols,
                )
                for h in range(3):
                    nc.vector.tensor_scalar_mul(
                        osb[:, half, 64 * h : 64 * h + 64],
                        outp[:, 256 * half + 65 * h : 256 * half + 65 * h + 64],
                        rec[:, 4 * half + h : 4 * half + h + 1],
                    )
                if has_kmask:
                    nc.vector.tensor_scalar_mul(
                        osb[:, half, :],
                        osb[:, half, :],
                        qpad_sb[:, 2 * n + half : 2 * n + half + 1],
                    )
                if n == NB - 1:
                    # final block: issue each half's DMA as soon as its
                    # scales land - shortens the end-of-kernel drain
                    nc.scalar.dma_start(
                        out_d[q0 + 128 * half : q0 + 128 * half + 128, :],
                        osb[:, half, :],
                    )
            if n != NB - 1:
                nc.sync.dma_start(
                    out_d[q0 : q0 + 256, :].rearrange(
                        "(h p) d -> p h d", p=128
                    ),
                    osb[:],
                )

        # Interleave: projection matmul groups between attention blocks keep
        # PE busy while exp/mask/PV chains drain on Act/Pool/DVE.
        emit_proj_qk(0)
        emit_proj_v(0)
        emit_proj_qk(1)
        emit_proj_v(1)
        emit_block(0)
        nb_next = 1
        for t in range(2, 7):
            emit_proj_qk(t)
            emit_block(nb_next)
            emit_proj_v(t)
            emit_block(nb_next + 1)
            nb_next += 2
        emit_proj_qk(7)
        emit_block(11)
        emit_proj_v(7, (0,), done=False)
        emit_block(12)
        emit_block(13)
        emit_proj_v(7, (2,), done=True)
        emit_block(14)
        emit_block(15)

    nc.compile()
    return nc


_prog_cache = {}


def _get_program(has_bias, has_kmask):
    key = (has_bias, has_kmask)
    if key not in _prog_cache:
        _prog_cache[key] = build_program(has_bias, has_kmask)
    return _prog_cache[key]


def _band_masks():
    """[L|L|U|U] multiplicative masks, [128, 512] bf16.

    L[r, j] = (j <= r) masks [mL1-tri | mL2]; U[r, j] = (j >= r) masks
    [mR2 | mR1-tri].
    """
    r = np.arange(128)[:, None]
    j = np.arange(128)[None, :]
    L = (j <= r).astype(np.float32)
    U = (j >= r).astype(np.float32)
    return np.concatenate([L, L, U, U], axis=1).astype(ml_dtypes.bfloat16)


def kernel(hidden_states, attention_mask, Wq, bq, Wk, bk, Wv, bv, _res=[None]):
    hidden_states = np.asarray(hidden_states, np.float32)
    attention_mask = np.asarray(attention_mask, np.float32)
    Wq, Wk, Wv = (np.asarray(w, np.float32) for w in (Wq, Wk, Wv))
    bq, bk, bv = (np.asarray(b_, np.float32) for b_ in (bq, bk, bv))

    scale = 1.0 / np.sqrt(DH)
    has_bias = bool(np.any(bq) or np.any(bk) or np.any(bv))
    has_kmask = bool(np.any(attention_mask < 0))

    hsT = [
        np.ascontiguousarray(hidden_states[b].T).astype(ml_dtypes.bfloat16)
        for b in range(B)
    ]
    masks = _band_masks()
    masked = attention_mask < 0  # [B, S]

    in_maps = []
    for core in range(N_CORES):
        b, h0 = core // 4, (core % 4) * HPC
        sl = slice(h0 * DH, (h0 + HPC) * DH)
        wq = Wq[:, sl] * scale
        wk = Wk[:, sl]
        wqkv = np.concatenate(
            [wq[:, 0:128], wk[:, 0:128], wq[:, 128:192], wk[:, 128:192],
             Wv[:, sl]],
            axis=1,
        ).astype(ml_dtypes.bfloat16)
        m = {
            "hsT": hsT[b],
            "wqkv": np.ascontiguousarray(wqkv),
            "masks": masks,
        }
        if has_bias:
            bq_s = bq[sl] * scale
            bk_s = bk[sl]
            m["bqkv"] = np.concatenate(
                [bq_s[0:128], bk_s[0:128], bq_s[128:192], bk_s[128:192],
                 bv[sl]]
            ).reshape(1, 576).astype(ml_dtypes.bfloat16)
        if has_kmask:
            keep = (~masked[b]).astype(np.float32).reshape(NKC, 128).T
            m["kpad"] = np.ascontiguousarray(keep)
            m["qpad"] = np.ascontiguousarray(keep)
        in_maps.append(m)

    nc = _get_program(has_bias, has_kmask)
    res = run_bass_kernel_spmd(nc, in_maps, list(range(N_CORES)))
    _res[0] = res

    out = np.empty((B, S, D), np.float32)
    for core in range(N_CORES):
        b, h0 = core // 4, (core % 4) * HPC
        out[b, :, h0 * DH : (h0 + HPC) * DH] = res.results[core]["out"]
    return out


# revision 24
# speedup vs baseline: 1.1680x; 1.1680x over previous
"""Longformer-style windowed self-attention for TRN2, 8-core SPMD.

Sharding: 24 (batch, head) pairs -> 3 heads per core (core c gets batch c//4,
heads (c%4)*3 .. +3). Each core computes QKV projections for its head slice,
windowed attention (block 256, window +-256), and writes its [4096, 192]
output channel slice. Host gathers slices into the full [2, 4096, 768] output.

All matmul inputs are bf16 (inputs/weights converted on host). Scores are
computed transposed ([keys, queries]); probs (exp'd scores) become the
stationary operand of the PV matmul, which therefore produces output directly
in [queries, head_dim] layout with a ones-column carrying the softmax
denominator - no PE transposes needed. Band-mask multiplies run on GpSimd,
exp on the scalar engine, PSUM evacuation + normalize scaling on DVE.
"""

import sys

for _p in ("/opt/trn_rl_repo", "/opt/pypackages"):
    if _p not in sys.path:
        sys.path.append(_p)

import numpy as np
import ml_dtypes
from contextlib import ExitStack

import concourse.bass as bass
import concourse.bacc as bacc
import concourse.mybir as mybir
import concourse.tile as tile
from concourse.bass_utils import run_bass_kernel_spmd

F32 = mybir.dt.float32
BF16 = mybir.dt.bfloat16
EXP = mybir.ActivationFunctionType.Exp
MUL = mybir.AluOpType.mult

B, S, D = 2, 4096, 768
H, DH = 12, 64
W = 256                 # one-sided window / query block size
NB = S // W             # 16 query blocks
NKC = S // 128          # 32 key chunks of 128
HPC = 3                 # heads per core
N_CORES = 8


def block_layout(n):
    """Score-PSUM column layout for query block n.

    Returns (pieces, maskop, ncols). pieces = [(m, qlo, qhi, col)]: key chunk
    m's scores for local queries [qlo, qhi) live at psum cols [col, col+qhi-qlo).
    maskop = (dst_col, width, src_col) multiplies pt[:, dst:dst+width] by
    msk[:, src:src+width] (msk = [L|L|U|U]). 256-wide pieces sit at byte
    offsets that never straddle a 2KB PSUM bank.
    """
    if n == 0:
        pieces = [(0, 0, 256, 0), (1, 0, 256, 256),
                  (3, 128, 256, 512), (2, 0, 256, 640)]
        maskop = (512, 256, 256)  # [mR2 | mR1 tri] *= [U|U]
        ncols = 896
    elif n == NB - 1:
        m0 = 2 * n
        pieces = [(m0, 0, 256, 0), (m0 - 1, 0, 256, 256),
                  (m0 - 2, 0, 128, 512), (m0 + 1, 0, 256, 640)]
        maskop = (384, 256, 0)    # [mL1 tri | mL2] *= [L|L]
        ncols = 896
    else:
        pieces = [(2 * n - 1, 0, 256, 0), (2 * n - 2, 0, 128, 256),
                  (2 * n + 3, 128, 256, 384), (2 * n + 2, 0, 256, 512),
                  (2 * n, 0, 256, 768), (2 * n + 1, 0, 256, 1024)]
        maskop = (128, 512, 0)    # [mL1 tri | mL2 | mR2 | mR1 tri] *= [L|L|U|U]
        ncols = 1280
    return pieces, maskop, ncols


def pv_chunks(pieces, half):
    """(m, pt_col) for key chunks fully covering query half [128h, 128h+128)."""
    q0, q1 = 128 * half, 128 * half + 128
    return [(m, col + q0 - qlo) for (m, qlo, qhi, col) in pieces
            if qlo <= q0 and q1 <= qhi]


def build_program(has_bias, has_kmask):
    nc = bacc.Bacc("TRN2", target_bir_lowering=False, debug=False,
                   num_devices=N_CORES)
    hsT_d = nc.declare_dram_parameter("hsT", [D, S], BF16, isOutput=False)
    w_d = nc.declare_dram_parameter("wqkv", [D, 576], BF16, isOutput=False)
    msk_d = nc.declare_dram_parameter("masks", [128, 512], BF16, isOutput=False)
    if has_bias:
        bqkv_d = nc.declare_dram_parameter("bqkv", [1, 576], BF16, isOutput=False)
    if has_kmask:
        kpad_d = nc.declare_dram_parameter("kpad", [128, NKC], F32, isOutput=False)
        qpad_d = nc.declare_dram_parameter("qpad", [128, NKC], F32, isOutput=False)
    out_d = nc.declare_dram_parameter("out", [S, HPC * DH], F32, isOutput=True)

    with tile.TileContext(nc) as tc, ExitStack() as ctx:
        const_p = ctx.enter_context(tc.tile_pool(name="const", bufs=1))
        hst_p = ctx.enter_context(tc.tile_pool(name="hst", bufs=3))
        qkt_p = ctx.enter_context(tc.tile_pool(name="qkt", bufs=1))
        vall_p = ctx.enter_context(tc.tile_pool(name="vall", bufs=1))
        pt_p = ctx.enter_context(tc.tile_pool(name="pt", bufs=4))
        wk_p = ctx.enter_context(tc.tile_pool(name="wk", bufs=4))
        ps_p = ctx.enter_context(tc.tile_pool(name="ps", bufs=2, space="PSUM"))
        sm_p = ctx.enter_context(tc.tile_pool(name="sm", bufs=2, space="PSUM"))

        # ---- constants / weights ----
        wsb = const_p.tile([128, 6, 576], BF16)
        w_r = w_d[:].rearrange("(c p) n -> p c n", p=128)

        hst_tiles = {}

        def dma_hst(t):
            hst = hst_p.tile([128, 6, 512], BF16)
            hst_tiles[t] = hst
            src = hsT_d[:].rearrange("(c p) s -> p c s", p=128)[
                :, :, 512 * t : 512 * t + 512
            ]
            if t == 0:  # split so the first projection group starts sooner
                nc.sync.dma_start(hst[:, 0:2, :], src[:, 0:2, :])
                nc.sync.dma_start(hst[:, 2:4, :], src[:, 2:4, :])
                nc.sync.dma_start(hst[:, 4:6, :], src[:, 4:6, :])
            else:
                nc.sync.dma_start(hst[:], src)

        nc.sync.dma_start(wsb[:, :, 0:128], w_r[:, :, 0:128])
        dma_hst(0)
        nc.sync.dma_start(wsb[:, :, 128:576], w_r[:, :, 128:576])
        msk_sb = const_p.tile([128, 512], BF16)
        nc.sync.dma_start(msk_sb[:], msk_d[:, :])
        dma_hst(1)
        if has_bias:
            bqkv_sb = const_p.tile([1, 576], BF16)
            nc.sync.dma_start(bqkv_sb[:], bqkv_d[:, :])
            ones_sb = const_p.tile([1, 512], BF16)
            nc.vector.memset(ones_sb[:], 1.0)
        if has_kmask:
            kpad_sb = const_p.tile([128, NKC], F32)
            nc.sync.dma_start(kpad_sb[:], kpad_d[:, :])
            qpad_sb = const_p.tile([128, NKC], F32)
            nc.sync.dma_start(qpad_sb[:], qpad_d[:, :])

        # PE warmup: dummy matmuls keep the tensor engine "busy" while the
        # first DMAs land, so the p-state ramp hits full clock before real
        # matmuls start. Inputs are never-written scratch; output is the
        # first sm-pool psum tile, freed immediately (no readers).
        warm_sb = const_p.tile([1, 512], BF16)
        nc.vector.memset(warm_sb[:], 0.0)
        warm_ps = sm_p.tile([128, 512], F32, space="PSUM", tag="sm")
        for _ in range(10):
            nc.tensor.matmul(
                warm_ps[:], warm_sb[0:1, 0:128], warm_sb[0:1, :],
                start=True, stop=True,
            )

        # qT/kT for head pair (A,B): A on partitions 0:64, B on 64:128
        qt_ab = qkt_p.tile([128, S], BF16)
        kt_ab = qkt_p.tile([128, S], BF16)
        # solo head C: base-0 tiles
        qt_c = qkt_p.tile([64, S], BF16)
        kt_c = qkt_p.tile([64, S], BF16)
        # v in [key, dh] layout: [128, key-chunk, (vA|1|vB|1|vC|1)]
        vall = vall_p.tile([128, NKC, 195], BF16)
        ones_cols = vall[:].rearrange("p m (h x) -> p m h x", h=3)[:, :, :, 64:65]
        nc.vector.memset(ones_cols, 1.0)

        def emit_proj_qk(t):
            s0 = 512 * t
            if t + 1 < 8:
                dma_hst(t + 1)
            hst = hst_tiles[t]
            for j in range(3):
                pp = sm_p.tile([128, 512], F32, space="PSUM", tag="sm")
                for c in range(6):
                    nc.tensor.matmul(
                        pp[:],
                        (wsb[:, c, 128 * j : 128 * j + 128]),
                        (hst[:, c, :]),
                        start=(c == 0),
                        stop=(c == 5 and not has_bias),
                    )
                if has_bias:
                    nc.tensor.matmul(
                        pp[:],
                        (bqkv_sb[0:1, 128 * j : 128 * j + 128]),
                        (ones_sb[0:1, :]),
                        start=False,
                        stop=True,
                    )
                if j == 0:
                    nc.vector.tensor_copy(qt_ab[:, s0 : s0 + 512], pp[:])
                elif j == 1:
                    nc.vector.tensor_copy(kt_ab[:, s0 : s0 + 512], pp[:])
                else:
                    nc.vector.tensor_copy(qt_c[:, s0 : s0 + 512], pp[0:64, :])
                    kcs = wk_p.tile([128, 512], BF16, name="kcs")
                    nc.vector.tensor_copy(kcs[64:128, :], pp[64:128, :])
                    nc.sync.dma_start(kt_c[:, s0 : s0 + 512], kcs[64:128, :])

        def emit_proj_v(t, groups=(0, 2), done=True):
            hst = hst_tiles.pop(t) if done else hst_tiles[t]
            for mm0 in groups:
                m = 4 * t + mm0
                pv = sm_p.tile([128, 512], F32, space="PSUM", tag="sm")
                for half, mm in enumerate((mm0, mm0 + 1)):
                    for c in range(6):
                        nc.tensor.matmul(
                            pv[:, 256 * half : 256 * half + 192],
                            (hst[:, c, 128 * mm : 128 * mm + 128]),
                            (wsb[:, c, 384:576]),
                            start=(c == 0),
                            stop=(c == 5 and not has_bias),
                        )
                    if has_bias:
                        nc.tensor.matmul(
                            pv[:, 256 * half : 256 * half + 192],
                            (ones_sb[0:1, 0:128]),
                            (bqkv_sb[0:1, 384:576]),
                            start=False,
                            stop=True,
                        )
                dst = vall[:, m : m + 2, :].rearrange(
                    "p m (h x) -> p m h x", h=3
                )[:, :, :, 0:64]
                src = pv[:].rearrange("p (m x) -> p m x", m=2)[
                    :, :, 0:192
                ].rearrange("p m (h x) -> p m h x", h=3)
                nc.vector.tensor_copy(dst, src)

        HEADS = (
            (lambda: kt_ab[0:64, :], lambda: qt_ab[0:64, :]),
            (lambda: kt_ab[64:128, :], lambda: qt_ab[64:128, :]),
            (lambda: kt_c[:, :], lambda: qt_c[:, :]),
        )

        def emit_block(n):
            pieces, maskop, ncols = block_layout(n)
            q0 = 256 * n
            pts = []
            for h, (ktf, qtf) in enumerate(HEADS):
                kt, qt = ktf(), qtf()
                ps = ps_p.tile([128, 1280], F32, space="PSUM", tag="ps")
                for m, qlo, qhi, col in pieces:
                    nc.tensor.matmul(
                        ps[:, col : col + qhi - qlo],
                        (kt[:, 128 * m : 128 * m + 128]),
                        (qt[:, q0 + qlo : q0 + qhi]),
                        start=True,
                        stop=True,
                    )
                pt = pt_p.tile([128, 1280], BF16, tag="pt")
                pts.append(pt)
                nc.scalar.activation(pt[:, 0:ncols], ps[:, 0:ncols], EXP)
                dcol, width, scol = maskop
                nc.vector.scalar_tensor_tensor(
                    pt[:, dcol : dcol + width],
                    pt[:, dcol : dcol + width],
                    1.0,
                    msk_sb[:, scol : scol + width],
                    MUL,
                    MUL,
                )
                if has_kmask:
                    for m, qlo, qhi, col in pieces:
                        nc.vector.tensor_scalar_mul(
                            pt[:, col : col + qhi - qlo],
                            pt[:, col : col + qhi - qlo],
                            kpad_sb[:, m : m + 1],
                        )

            # PV: out[q, dh] = pt(chunk).T @ [v|1]; col 64 of each head's rhs
            # slice is the ones column carrying the softmax denominator.
            outp = sm_p.tile([128, 512], F32, space="PSUM", tag="sm")
            dcol, width, _ = maskop
            for h, pt in enumerate(pts):
                for half in (0, 1):
                    chunks = pv_chunks(pieces, half)
                    # unmasked chunks first: their matmuls only depend on the
                    # exp, so PV starts while the mask op is still running
                    chunks.sort(
                        key=lambda mp: not (
                            mp[1] + 128 <= dcol or mp[1] >= dcol + width
                        )
                    )
                    for ci, (m, pcol) in enumerate(chunks):
                        nc.tensor.matmul(
                            outp[:, 256 * half + 65 * h : 256 * half + 65 * h + 65],
                            (pt[:, pcol : pcol + 128]),
                            (vall[:, m, 65 * h : 65 * h + 65]),
                            start=(ci == 0),
                            stop=(ci == len(chunks) - 1),
                        )

            # single Act copy evacuates PV psum fast (frees the shared sm
            # pool for the next projection group); recip/scales read SBUF
            src = outp
            if n != NB - 1:
                ocp = wk_p.tile([128, 512], F32, name="ocp")
                nc.scalar.copy(ocp[:, 0:451], outp[:, 0:451])
                src = ocp
            rec = wk_p.tile([128, 8], F32, name="rec")
            osb = wk_p.tile([128, 2, 192], F32, name="osb")
            for half in (0, 1):
                dcols = src[:, 256 * half : 256 * half + 195].rearrange(
                    "p (i x) -> p i x", x=65
                )[:, :, 64:65]
                nc.vector.reciprocal(
                    rec[:, 4 * half : 4 * half + 3].rearrange(
                        "p (i x) -> p i x", x=1
                    ),
                    dcols,
                )
                for h in range(3):
                    nc.vector.tensor_scalar_mul(
                        osb[:, half, 64 * h : 64 * h + 64],
                        src[:, 256 * half + 65 * h : 256 * half + 65 * h + 64],
                        rec[:, 4 * half + h : 4 * half + h + 1],
                    )
                if has_kmask:
                    nc.vector.tensor_scalar_mul(
                        osb[:, half, :],
                        osb[:, half, :],
                        qpad_sb[:, 2 * n + half : 2 * n + half + 1],
                    )
                if n == NB - 1:
                    # final block: issue each half's DMA as soon as its
                    # scales land - shortens the end-of-kernel drain
                    nc.scalar.dma_start(
                        out_d[q0 + 128 * half : q0 + 128 * half + 128, :],
                        osb[:, half, :],
                    )
            if n != NB - 1:
                nc.sync.dma_start(
                    out_d[q0 : q0 + 256, :].rearrange(
                        "(h p) d -> p h d", p=128
                    ),
                    osb[:],
                )

        # Interleave: projection matmul groups between attention blocks keep
        # PE busy while exp/mask/PV chains drain on Act/Pool/DVE.
        emit_proj_qk(0)
        emit_proj_v(0)
        emit_proj_qk(1)
        emit_proj_v(1)
        emit_block(0)
        nb_next = 1
        for t in range(2, 7):
            emit_proj_qk(t)
            emit_block(nb_next)
            emit_proj_v(t)
            emit_block(nb_next + 1)
            nb_next += 2
        emit_proj_qk(7)
        emit_block(11)
        emit_proj_v(7, (0,), done=False)
        emit_block(12)
        emit_block(13)
        emit_proj_v(7, (2,), done=True)
        emit_block(14)
        emit_block(15)

    nc.compile()
    return nc


_prog_cache = {}


def _get_program(has_bias, has_kmask):
    key = (has_bias, has_kmask)
    if key not in _prog_cache:
        _prog_cache[key] = build_program(has_bias, has_kmask)
    return _prog_cache[key]


def _band_masks():
    """[L|L|U|U] multiplicative masks, [128, 512] bf16.

    L[r, j] = (j <= r) masks [mL1-tri | mL2]; U[r, j] = (j >= r) masks
    [mR2 | mR1-tri].
    """
    r = np.arange(128)[:, None]
    j = np.arange(128)[None, :]
    L = (j <= r).astype(np.float32)
    U = (j >= r).astype(np.float32)
    return np.concatenate([L, L, U, U], axis=1).astype(ml_dtypes.bfloat16)


def kernel(hidden_states, attention_mask, Wq, bq, Wk, bk, Wv, bv, _res=[None]):
    hidden_states = np.asarray(hidden_states, np.float32)
    attention_mask = np.asarray(attention_mask, np.float32)
    Wq, Wk, Wv = (np.asarray(w, np.float32) for w in (Wq, Wk, Wv))
    bq, bk, bv = (np.asarray(b_, np.float32) for b_ in (bq, bk, bv))

    scale = 1.0 / np.sqrt(DH)
    has_bias = bool(np.any(bq) or np.any(bk) or np.any(bv))
    has_kmask = bool(np.any(attention_mask < 0))

    hsT = [
        np.ascontiguousarray(hidden_states[b].T).astype(ml_dtypes.bfloat16)
        for b in range(B)
    ]
    masks = _band_masks()
    masked = attention_mask < 0  # [B, S]

    in_maps = []
    for core in range(N_CORES):
        b, h0 = core // 4, (core % 4) * HPC
        sl = slice(h0 * DH, (h0 + HPC) * DH)
        wq = Wq[:, sl] * scale
        wk = Wk[:, sl]
        wqkv = np.concatenate(
            [wq[:, 0:128], wk[:, 0:128], wq[:, 128:192], wk[:, 128:192],
             Wv[:, sl]],
            axis=1,
        ).astype(ml_dtypes.bfloat16)
        m = {
            "hsT": hsT[b],
            "wqkv": np.ascontiguousarray(wqkv),
            "masks": masks,
        }
        if has_bias:
            bq_s = bq[sl] * scale
            bk_s = bk[sl]
            m["bqkv"] = np.concatenate(
                [bq_s[0:128], bk_s[0:128], bq_s[128:192], bk_s[128:192],
                 bv[sl]]
            ).reshape(1, 576).astype(ml_dtypes.bfloat16)
        if has_kmask:
            keep = (~masked[b]).astype(np.float32).reshape(NKC, 128).T
            m["kpad"] = np.ascontiguousarray(keep)
            m["qpad"] = np.ascontiguousarray(keep)
        in_maps.append(m)

    nc = _get_program(has_bias, has_kmask)
    res = run_bass_kernel_spmd(nc, in_maps, list(range(N_CORES)))
    _res[0] = res

    out = np.empty((B, S, D), np.float32)
    for core in range(N_CORES):
        b, h0 = core // 4, (core % 4) * HPC
        out[b, :, h0 * DH : (h0 + HPC) * DH] = res.results[core]["out"]
    return out
